# revision 51
# baseline (speedup 1.0000x reference)
"""AnchorwCrossEntropyLoss (debiased Sinkhorn anchor loss) — TRN2 Bass kernel.

Math note (why the device kernel is tiny):
The reference computes a debiased Sinkhorn divergence between, per sample b,
a degenerate cloud of M identical copies of logits[b] and the M anchor rows.
Because the x-cloud points are identical per sample:
  * f_aa is identically 0,
  * g_bb only involves anchor-anchor distances -> sample-independent, host-computable,
  * f_ba is a per-sample scalar and (g_ab - dxy) stays constant across anchors,
    which collapses the whole symmetric eps-scaling loop in closed form.
The surviving value is    dist[b] = mean_j ||x_b - a_j||  -  mean_i(g_bb_n[i])
(verified < 1e-7 rel err against the full reference).  The device work is the
masked mean of per-sample anchor-distance means; the tiny (21,) g_bb recursion
and the eps schedule (both O(m^2 * n_iters) ~ 5k flops) run on host, exactly as
the reference itself computes the diameter/eps schedule on host.

Anchors are the fixed set_anchors matrix diag(+5)/off(-5), so
  ||x_b - a_j||^2 = sum_k (x_bk^2 + 10 x_bk) + 525 - 20 * x_bj
Two DMA chunks of 16 samples/partition: x0 on the scalar HWDGE ring (its
preamble finishes first), x1 alone on the sync ring, the tiny label DMA
behind x0.  Per chunk: square (chunk0 as x^2+10x on DVE so the pipeline
starts right at the x0 land; chunk1 as (x+5)^2 on ACT, bf16 out, both
tables pre-warmed behind the DMA issues), segmented reduce (DVE, shared
base tile pins the scheduler to pipeline order), fused -20x+base (DVE,
bf16 out), Sqrt (ACT, bf16), and a fused mask-multiply-accumulate (DVE,
f32 accum).  Labels are cast on GpSimd; the mask compare sits between the
phase-1 and masked passes in the in-order DVE queue so its late label
input can't stall the stream.  The 128-partition partials are collapsed by
a ones-matmul on TensorE, copied PSUM->SBUF on DVE, and a single-packet
HWDGE DMA on sync ships (1, n_chunks) floats out.  The kernel tail
replaces stock Tile's drain+barrier+clears+barrier with a one-wait gate on
the out-DMA completion + one all-engine barrier + gpsimd-local sem clears.
"""

import os
import sys

import ml_dtypes
import numpy as np

for _p in ("/opt/trn_rl_repo",):
    if _p not in sys.path and os.path.isdir(_p):
        sys.path.append(_p)


def _ensure_ntff_hook():
    """The agent image lacks antenv.axon_hooks; shim it so trace=True works."""
    import types
    try:
        import antenv.axon_hooks  # noqa: F401
        return
    except ImportError:
        pass
    try:
        import antenv
        from trn_agent_boot.trn_boot import _ntff_profile_via_ctypes
        mod = types.ModuleType("antenv.axon_hooks")
        _hook = [None]
        mod.set_axon_ntff_profile_hook = lambda h: _hook.__setitem__(0, h)
        mod.get_axon_ntff_profile_hook = lambda: _hook[0]
        sys.modules["antenv.axon_hooks"] = mod
        antenv.axon_hooks = mod
        mod.set_axon_ntff_profile_hook(
            _ntff_profile_via_ctypes("/opt/axon/libaxon_pjrt.so"))
    except Exception:
        pass

NUM_CLASSES = 20
M = NUM_CLASSES + 1          # 21
BLUR = 0.1
SCALING = 0.5
ANCHOR_WEIGHT = 0.1
LOSS_WEIGHT = 1.0
N_ROIS = 32768
N_CORES = 8
N_SH = N_ROIS // N_CORES     # 4096 rois per core
P = 128                      # partitions
R = N_SH // P                # 32 samples per partition

# knobs (env-tunable for experiments)
# chunk sizes in units of samples-per-partition (must sum to R=32).  A small
# first chunk lets compute start as soon as its DMA lands; bigger later
# chunks amortize per-instruction overheads.
CHUNK_SIZES = [int(s) for s in
               os.environ.get("KERNEL_CHUNK_SIZES", "16,16").split(",")]
N_CHUNKS = len(CHUNK_SIZES)

LAST_EXEC_NS = None
LAST_RESULTS = None

_built = {}


def _default_anchors() -> np.ndarray:
    return np.where(np.eye(M, dtype=bool), 5.0, -5.0).astype(np.float32)


def _eps_schedule(diameter: float, blur: float, scaling: float) -> list:
    return ([diameter]
            + [float(np.exp(e))
               for e in np.arange(np.log(diameter), np.log(blur), np.log(scaling))]
            + [blur])


def _host_gbb_mean(cls_score: np.ndarray, anchors: np.ndarray) -> float:
    """mean_i(g_bb_n[i]) of the reference, computed exactly on host (f64)."""
    pts = np.concatenate([np.asarray(cls_score), np.asarray(anchors)], axis=0)
    diameter = float(np.linalg.norm(pts.max(axis=0) - pts.min(axis=0)))
    eps_list = _eps_schedule(diameter, BLUR, SCALING)

    a = np.asarray(anchors, dtype=np.float64)
    A = np.sqrt(((a[:, None, :] - a[None, :, :]) ** 2).sum(-1))  # (M, M)
    bl = -np.log(M)

    def lse(v):  # rowwise logsumexp over last axis
        mx = v.max(axis=-1, keepdims=True)
        return (mx + np.log(np.exp(v - mx).sum(axis=-1, keepdims=True)))[..., 0]

    eps0 = eps_list[0]
    g = -eps0 * lse(bl - A / eps0)
    for eps in eps_list:
        gt = -eps * lse(bl + g[None, :] / eps - A / eps)
        g = 0.5 * (g + gt)
    blur = eps_list[-1]
    g_n = -blur * lse(bl + g[None, :] / blur - A / blur)
    return float(g_n.mean())


def _make_tile_context_cls():
    """TileContext with a lightweight kernel tail.

    Stock Tile ends with drain + all-engine barrier + sem clears + second
    all-engine barrier (~3-5us of EVSEM ping-pong).  All we actually need for
    a correct, re-executable NEFF is: one instruction that waits until every
    tracked semaphore reached its final value, then the gpsimd sem clears
    (same engine -> program order).  Every engine then simply ends; the
    runtime completes the NEFF when all engines halt.
    """
    import concourse.tile as tile
    from concourse.vector_clock import ScopedClock

    tail_mode = os.environ.get("KERNEL_TAIL", "early")

    class FastEndTileContext(tile.TileContext):
        def _drain_and_barrier(self, tick_clock, wait_clock):
            # The body's data deps give a total order that ends at the
            # gpsimd-issued out-DMA: every other semaphore reaches its final
            # value strictly before that DMA can issue.  So the tail is one
            # gpsimd gate waiting on just the out-DMA completion semaphore,
            # then gpsimd-local sem clears.  No all-engine barrier: each
            # engine simply ends; the runtime completes the NEFF when all
            # engines halt.
            nc = self.nc
            if tail_mode == "early":
                # barrier FIRST: engines sync while the out-DMA is still in
                # flight (its ~900ns HBM receipt hides the ~280ns barrier);
                # the gate then waits the receipt, and gpsimd program order
                # still puts the clears after both.
                nc.all_engine_barrier()
            gate = nc.gpsimd.nop(nofuse=True, hint="tail_gate")
            wait_clock.add_sem_waits(
                gate.ins, ScopedClock({None: tick_clock.global_clock}))
            out_dma = getattr(nc, "_tail_dma_inst", None)
            if out_dma is not None:
                upd_ids = {u.id for u in out_dma.ins.sync_info.on_update}
                assert upd_ids, "out-DMA has no completion sem"
                si = gate.ins.sync_info
                kept = [w for w in si.on_wait if w.id in upd_ids]
                assert kept, "gate lost the out-DMA completion wait"
                si.on_wait = kept
            # One true all-engine barrier (with per-engine drains) before the
            # sem/DMA-ring clears: every sequencer and DGE ring is quiesced
            # when the reset executes, which re-execution and back-to-back
            # NEFF loads rely on.  (Stock Tile additionally does a global-
            # wait drain before and a second barrier after; both are
            # redundant here: the gate already proved the last DMA landed,
            # and after the clears only gpsimd has instructions left.)
            if tail_mode == "safe":
                nc.all_engine_barrier()
            popped = nc._tile_sem_poison_stack.pop()
            assert popped is self._sem_poison
            nc.clear_and_free_semaphores(list(self.sems.allocated().values()))

    return FastEndTileContext


def _build_nc(chunk_sizes=None):
    import concourse.tile as tile
    from concourse import bacc, mybir

    f32 = mybir.dt.float32
    bf16 = mybir.dt.bfloat16
    i32 = mybir.dt.int32
    AF = mybir.ActivationFunctionType
    OP = mybir.AluOpType
    AX = mybir.AxisListType

    CHUNK_SIZES = list(chunk_sizes) if chunk_sizes else globals()["CHUNK_SIZES"]
    N_CHUNKS = len(CHUNK_SIZES)
    assert sum(CHUNK_SIZES) == R
    offs = [sum(CHUNK_SIZES[:i]) for i in range(N_CHUNKS)]

    # compute chunks may subdivide the DMA chunks: a smaller LAST compute
    # chunk shortens the serial d2->sqrt->masked tail.  Each must lie
    # inside one DMA chunk.
    csplit = [int(s) for s in
              os.environ.get("KERNEL_CSPLIT",
                             ",".join(map(str, CHUNK_SIZES))).split(",")]
    assert sum(csplit) == R
    comp = []
    pos = 0
    for cs in csplit:
        di = max(i for i in range(N_CHUNKS) if offs[i] <= pos)
        assert pos + cs <= offs[di] + CHUNK_SIZES[di], (
            f"compute chunk {pos}:{pos + cs} spans DMA chunks")
        comp.append((di, pos, cs))
        pos += cs

    # the sqrt+masked stage may subdivide further: smaller LAST pieces
    # shorten the serial d2->sqrt->masked endgame without paying the extra
    # sq/reduce/d2 fixed costs that make full compute-splits lose.
    msplit = [int(s) for s in
              os.environ.get("KERNEL_MSPLIT",
                             ",".join(str(cs) for _, _, cs in comp))
              .split(",")]
    assert sum(msplit) == R
    mpieces = []
    pos = 0
    for ms in msplit:
        ci = max(i for i in range(len(comp)) if comp[i][1] <= pos)
        assert pos + ms <= comp[ci][1] + comp[ci][2], (
            f"masked piece {pos}:{pos + ms} spans compute chunks")
        mpieces.append((ci, pos, ms))
        pos += ms
    NCC = len(mpieces)

    OUT_MODE = os.environ.get("KERNEL_OUT", "matmul")  # matmul | direct

    # Bass.__init__ registers const APs (4 memsets) + an all-engine barrier
    # (~0.8us on silicon).  This kernel never reads those const APs (every
    # activation bias is an explicit tile), so elide the barrier.
    import concourse.bass as bass_mod
    skip_init_barrier = os.environ.get("KERNEL_SKIP_INIT_BARRIER", "1") == "1"
    orig_barrier = bass_mod.Bass.all_engine_barrier
    if skip_init_barrier:
        bass_mod.Bass.all_engine_barrier = lambda self, **kw: None
    try:
        nc = bacc.Bacc(None, target_bir_lowering=False)
    finally:
        bass_mod.Bass.all_engine_barrier = orig_barrier
    # the logits can ride to the device as bf16 (host converts): halves the
    # DMA transfer and makes the chunk-0 square all-bf16 unit-stride.  The
    # 0.4% input quantization is far inside the 2e-2 tolerance.
    XBF = os.environ.get("KERNEL_XDT", "f32") == "bf16"
    x_d = nc.declare_dram_parameter("cls_score", [N_SH, M],
                                    bf16 if XBF else f32, isOutput=False)
    l_d = nc.declare_dram_parameter("label", [N_SH], i32, isOutput=False)
    out_rows = P if OUT_MODE == "direct" else 1
    out_d = nc.declare_dram_parameter("out", [out_rows, NCC], f32,
                                      isOutput=True)

    # partition p owns rows [R*p, R*(p+1)) -> contiguous 84*R bytes per partition
    x_f = x_d.rearrange("(p r) m -> p (r m)", p=P)   # (128, R*M)
    l_v = l_d.rearrange("(p r) -> p r", p=P)

    tc_cls = (_make_tile_context_cls()
              if os.environ.get("KERNEL_FAST_END", "1") == "1"
              else tile.TileContext)
    with tc_cls(nc) as tc:
        with (
            tc.tile_pool(name="io", bufs=2) as io_pool,
            tc.tile_pool(name="tmp", bufs=2) as tmp_pool,
            tc.tile_pool(name="acc", bufs=1) as acc_pool,
            tc.tile_pool(name="ps", bufs=1, space="PSUM") as psum_pool,
        ):
            outt = acc_pool.tile([P, NCC], f32)
            # ACT executes ONLY Sqrt ops: one lazy table load, triggered by
            # the early warm op and hidden under the input-DMA wait.  Consts
            # come from gpsimd memsets (explicit bias tiles keep the Bass
            # const-AP machinery and its init barrier unused).
            c525 = acc_pool.tile([P, 1], f32)
            nc.gpsimd.memset(c525[:], 525.0)
            ones = acc_pool.tile([P, 1], f32)
            nc.gpsimd.memset(ones[:], 1.0)
            c5 = acc_pool.tile([P, 1], f32)
            nc.gpsimd.memset(c5[:], 5.0)
            c0 = acc_pool.tile([P, 1], f32)
            nc.gpsimd.memset(c0[:], 0.0)
            # dedicated input tiles per chunk: DMAs never reuse slots, so each
            # DMA carries zero sync waits (HW DMA-direct allows only one).
            # All DMA issues go FIRST in each engine's queue -- warm ops
            # (with their ~1.5us blocking table loads) come after.
            xts = [io_pool.tile([P, CHUNK_SIZES[c] * M],
                                bf16 if XBF else f32,
                                tag=f"xt{c}", name=f"xt{c}")
                   for c in range(N_CHUNKS)]
            lt_all = io_pool.tile([P, R], i32, name="lt_all")
            layout = os.environ.get("KERNEL_DMA_LAYOUT", "v2")
            if os.environ.get("KERNEL_DMA_WARMUP", "0") == "1":
                # tiny dummy DMAs absorb any cold-ring start cost before the
                # real input DMAs ride the same rings.
                dw0 = io_pool.tile([1, 1], i32, name="dw0")
                dw1 = io_pool.tile([1, 1], i32, name="dw1")
                nc.scalar.dma_start(dw0[:], l_d.rearrange("(a b) -> a b", a=1)
                                    [:1, :1])
                nc.sync.dma_start(dw1[:], l_d.rearrange("(a b) -> a b", a=1)
                                  [:1, :1])
            if layout == "v2":
                # scalar's preamble finishes first -> x0 there for the
                # earliest possible land; x1 alone on the sync ring (it
                # gates the second half of the pipeline); the tiny label
                # DMA rides scalar behind x0, still well before its ~12us
                # consumer.
                nc.scalar.dma_start(
                    xts[0][:], x_f[:, :CHUNK_SIZES[0] * M])
                for c in range(1, N_CHUNKS):
                    nc.sync.dma_start(
                        xts[c][:],
                        x_f[:, offs[c] * M:(offs[c] + CHUNK_SIZES[c]) * M])
                nc.scalar.dma_start(lt_all[:], l_v)
            else:  # v1: label on scalar, all x chunks on sync
                nc.scalar.dma_start(lt_all[:], l_v)
                for c in range(N_CHUNKS):
                    nc.sync.dma_start(
                        xts[c][:],
                        x_f[:, offs[c] * M:(offs[c] + CHUNK_SIZES[c]) * M])

            if os.environ.get("KERNEL_WARM", "1") == "1":
                # warm BOTH tables with the real in/out dtypes, Square first
                # (needed first); the loads hide under the x-DMA wait.
                warm_b = acc_pool.tile([P, 1], bf16)
                nc.gpsimd.memset(warm_b[:], 525.0)
                warm_sq = acc_pool.tile([P, 1], bf16)
                nc.scalar.activation(warm_sq[:],
                                     (warm_b if XBF else c525)[:],
                                     AF.Square, bias=c5[:])
                warm2 = acc_pool.tile([P, 1], bf16)
                nc.scalar.activation(warm2[:], warm_b[:], AF.Sqrt,
                                     bias=c0[:])

            # ONE base tile shared by all chunks: chunk c+1's reduce carries a
            # WAR dependency on chunk c's d2 (its reader), which pins the DVE
            # stream to pipeline order.  Without it the Tile scheduler may put
            # reduce1 (gated on the late x1 DMA) ahead of d2_0 and idle the
            # DVE for ~2us.
            base_sh = tmp_pool.tile([P, max(cs for _, _, cs in comp)], f32,
                                    name="base")

            def T(shape, nm, c, dt=f32):
                return tmp_pool.tile(shape, dt, tag=f"{nm}{c}",
                                     name=f"{nm}{c}")

            FOLD = os.environ.get("KERNEL_FOLD", "0") == "1"
            if FOLD:
                # mask folded into the sqrt argument: mb = vmask*(base+K+200)
                # makes invalid samples' d2-200 strictly negative, the ACT
                # Sqrt clamps them to 0, and the sqrt's own accum_out sums
                # each chunk -- the two full-size masked DVE passes vanish.
                cn200 = acc_pool.tile([P, 1], f32)
                nc.gpsimd.memset(cn200[:], -200.0)
                mb_sh = tmp_pool.tile([P, R], f32, name="mb")

            # phase 1 per chunk: sq -> segmented reduce -> d2 -> sqrt.
            ds = []
            for c, (di, start, RC) in enumerate(comp):
                W = RC * M
                xt = xts[di][:, (start - offs[di]) * M:
                             (start - offs[di] + RC) * M]
                # sq on ACT = (x+5)^2 (bf16 out; folds +525 into base so the
                # Sqrt bias is 0).  Chunk 0 instead squares on the DVE
                # (x^2+10x, Sqrt bias 525): it starts the pipeline right at
                # the x0-DMA land instead of behind ACT's ~3.2us two-table
                # warm chain, which then finishes hidden under chunk 0.
                sq0_dve = (c == 0 and
                           os.environ.get("KERNEL_SQ0", "dve") == "dve")
                sq = T([P, W], "sq", c, bf16)
                if sq0_dve:
                    nc.vector.scalar_tensor_tensor(
                        sq[:], in0=xt, scalar=10.0, in1=xt,
                        op0=OP.add, op1=OP.mult)
                else:
                    nc.scalar.activation(sq[:], xt, AF.Square, bias=c5[:])
                base = base_sh[:, :RC]
                nc.vector.reduce_sum(
                    base, sq[:].rearrange("p (r m) -> p r m", m=M),
                    axis=AX.X)
                if FOLD:
                    if c == 0:
                        # mask prep emitted AFTER sq0/reduce0 so it can't
                        # stall the in-order DVE queue ahead of the x0 land;
                        # its label inputs are ready by ~10us anyway.
                        labf = tmp_pool.tile([P, R], f32, name="labf")
                        nc.gpsimd.tensor_copy(labf[:], lt_all[:])
                        vmask = tmp_pool.tile([P, R], f32, name="vmask")
                        nc.vector.tensor_scalar(
                            vmask[:], labf[:], 20.0, None, OP.not_equal)
                    K = 525.0 if sq0_dve else 0.0
                    mb = mb_sh[:, start:start + RC]
                    nc.vector.scalar_tensor_tensor(
                        mb, in0=base, scalar=K + 200.0,
                        in1=vmask[:, start:start + RC],
                        op0=OP.add, op1=OP.mult)
                    d2 = T([P, W], "d2", c, bf16)
                    nc.vector.scalar_tensor_tensor(
                        d2[:].rearrange("p (r m) -> p r m", m=M),
                        in0=xt.rearrange("p (r m) -> p r m", m=M),
                        scalar=-20.0,
                        in1=mb.unsqueeze(2).broadcast_to((P, RC, M)),
                        op0=OP.mult, op1=OP.add)
                    d = T([P, W], "d", c, bf16)
                    nc.scalar.activation(d[:], d2[:], AF.Sqrt,
                                         bias=cn200[:],
                                         accum_out=outt[:, c:c + 1])
                    continue
                # d2 = -20*x + base' (bf16 out); with bf16 x the base
                # broadcast needs a tiny bf16 cast first (same-dtype reads)
                if XBF:
                    baseb = T([P, RC], "baseb", c, bf16)
                    nc.vector.tensor_copy(baseb[:], base)
                    b_in1 = baseb[:]
                else:
                    b_in1 = base
                d2 = T([P, W], "d2", c, bf16)
                nc.vector.scalar_tensor_tensor(
                    d2[:].rearrange("p (r m) -> p r m", m=M),
                    in0=xt.rearrange("p (r m) -> p r m", m=M),
                    scalar=-20.0,
                    in1=b_in1.unsqueeze(2).broadcast_to((P, RC, M)),
                    op0=OP.mult, op1=OP.add)
                ds.append((d2, sq0_dve, start))

            if not FOLD:
                # mask prep: the DVE compare is 150ns that must slot into
                # the saturated DVE stream somewhere before masked0.  The
                # scheduler likes to put it BEFORE d2_0 (delaying the whole
                # critical chain); writing it into a bitcast alias of the
                # x0 tile -- whose last reader IS d2_0 -- adds a WAR edge
                # that pins it into the natural DVE idle gap right after
                # d2_0 instead.
                labf = tmp_pool.tile([P, R], f32, name="labf")
                nc.gpsimd.tensor_copy(labf[:], lt_all[:])
                if os.environ.get("KERNEL_VTRICK", "0") == "1":
                    vmask = xts[0][:, :R // 2].bitcast(bf16)
                else:
                    vmask = tmp_pool.tile([P, R], bf16, name="vmask")[:]
                veng = (nc.gpsimd if os.environ.get("KERNEL_VMASK", "dve")
                        == "gpsimd" else nc.vector)
                veng.tensor_scalar(
                    vmask, labf[:], 20.0, None, OP.not_equal)

                # phase 2 per piece: sqrt then fused mask-mult + accumulate
                for p_i, (ci, pstart, PRC) in enumerate(mpieces):
                    PW = PRC * M
                    d2, sq0_dve, cstart = ds[ci]
                    dsl = slice((pstart - cstart) * M,
                                (pstart - cstart + PRC) * M)
                    d = T([P, PW], "d", p_i, bf16)
                    nc.scalar.activation(d[:], d2[:, dsl], AF.Sqrt,
                                         bias=(c525 if sq0_dve else c0)[:])
                    masked = T([P, PW], "masked", p_i, bf16)
                    if os.environ.get("KERNEL_MSWAP", "0") == "1":
                        # broadcast operand on rd0 (re-reads one word), the
                        # unit-stride d streams on rd1 -- may unlock the 2x
                        # packed mode that a broadcast on rd1 blocks.
                        nc.vector.scalar_tensor_tensor(
                            masked[:].rearrange("p (r m) -> p r m", m=M),
                            in0=vmask[:, pstart:pstart + PRC].unsqueeze(2)
                            .broadcast_to((P, PRC, M)),
                            scalar=1.0,
                            in1=d[:].rearrange("p (r m) -> p r m", m=M),
                            op0=OP.mult, op1=OP.mult,
                            accum_out=outt[:, p_i:p_i + 1])
                    else:
                        nc.vector.scalar_tensor_tensor(
                            masked[:].rearrange("p (r m) -> p r m", m=M),
                            in0=d[:].rearrange("p (r m) -> p r m", m=M),
                            scalar=1.0,
                            in1=vmask[:, pstart:pstart + PRC].unsqueeze(2)
                            .broadcast_to((P, PRC, M)),
                            op0=OP.mult, op1=OP.mult,
                            accum_out=outt[:, p_i:p_i + 1])

            if OUT_MODE == "direct":
                # ship the [128, C] per-partition partials straight out on
                # the (idle, warm-ring) sync engine; the host sums them.
                nc._tail_dma_inst = nc.sync.dma_start(out_d[:], outt[:])
            elif OUT_MODE == "psum":
                # matmul collapse, then DMA straight from PSUM on sync --
                # skips the PSUM->SBUF copy and the slow scalar-ring issue.
                pr = psum_pool.tile([1, NCC], f32)
                nc.tensor.matmul(pr[:], ones[:], outt[:])
                nc._tail_dma_inst = nc.sync.dma_start(out_d[:], pr[:])
            else:
                # collapse partitions on the (otherwise idle) TensorE:
                # ones^T @ outt -> (1, C) in PSUM, single-descriptor DMA.
                pr = psum_pool.tile([1, NCC], f32)
                if os.environ.get("KERNEL_MMSPLIT", "1") == "1":
                    # per-column matmuls: col 0 runs early (its accumulator
                    # lands right after masked0), only the last column's
                    # matmul stays in the serial tail.
                    for c in range(NCC):
                        nc.tensor.matmul(pr[:, c:c + 1], ones[:],
                                         outt[:, c:c + 1])
                else:
                    nc.tensor.matmul(pr[:], ones[:], outt[:])
                prs = acc_pool.tile([1, NCC], f32)
                if os.environ.get("KERNEL_PSCOPY", "vector") == "vector":
                    nc.vector.tensor_copy(prs[:], pr[:])
                else:
                    nc.scalar.copy(prs[:], pr[:])
                eng = (nc.sync if os.environ.get("KERNEL_OUT_ENG", "sync")
                       == "sync" else nc.scalar)
                sp = os.environ.get("KERNEL_OUT_SP", "1") == "1"
                nc._tail_dma_inst = eng.dma_start(out_d[:], prs[:],
                                                  single_packet=sp)
    nc.finalize()
    return nc


def _get_built(chunk_sizes=None):
    cfg = tuple(chunk_sizes) if chunk_sizes else tuple(CHUNK_SIZES)
    key = (cfg, os.environ.get("KERNEL_TAIL", "early"),
           os.environ.get("KERNEL_FAST_END", "1"),
           os.environ.get("KERNEL_WARM", "1"),
           os.environ.get("KERNEL_OUT", "matmul"),
           os.environ.get("KERNEL_OUT_ENG", "sync"),
           os.environ.get("KERNEL_DMA_LAYOUT", "v2"),
           os.environ.get("KERNEL_PSCOPY", "vector"),
           os.environ.get("KERNEL_DMA_WARMUP", "0"),
           os.environ.get("KERNEL_SQ0", "dve"),
           os.environ.get("KERNEL_OUT_SP", "1"),
           os.environ.get("KERNEL_CSPLIT", ""),
           os.environ.get("KERNEL_VMASK", "dve"),
           os.environ.get("KERNEL_FOLD", "0"),
           os.environ.get("KERNEL_MSPLIT", ""),
           os.environ.get("KERNEL_VTRICK", "0"),
           os.environ.get("KERNEL_XDT", "f32"),
           os.environ.get("KERNEL_MSWAP", "0"),
           os.environ.get("KERNEL_MMSPLIT", "1"))
    if key not in _built:
        _built[key] = _build_nc(cfg)
    return _built[key]


def kernel(cls_score: np.ndarray, anchors: np.ndarray = None,
           label: np.ndarray = None, _chunk_sizes=None) -> np.ndarray:
    global LAST_EXEC_NS, LAST_RESULTS
    from concourse.bass_utils import run_bass_kernel_spmd

    cls_score = np.ascontiguousarray(np.asarray(cls_score, dtype=np.float32))
    label = np.ascontiguousarray(np.asarray(label, dtype=np.int32))
    if anchors is None:
        anchors = _default_anchors()
    anchors = np.asarray(anchors, dtype=np.float32)
    assert cls_score.shape == (N_ROIS, M) and label.shape == (N_ROIS,)

    gbb_mean = _host_gbb_mean(cls_score, anchors)

    nc = _get_built(_chunk_sizes)
    in_maps = []
    for i in range(N_CORES):
        sl = slice(i * N_SH, (i + 1) * N_SH)
        in_maps.append({
            "cls_score": np.ascontiguousarray(
                cls_score[sl].astype(ml_dtypes.bfloat16)
                if os.environ.get("KERNEL_XDT", "f32") == "bf16"
                else cls_score[sl]),
            "label": np.ascontiguousarray(label[sl]),
        })

    trace = (os.environ.get("KERNEL_TRACE", "0") == "1"
             or bool(os.environ.get("BASS_TRACE")))
    if trace:
        _ensure_ntff_hook()
    res = run_bass_kernel_spmd(nc, in_maps, core_ids=list(range(N_CORES)),
                               trace=trace)
    LAST_EXEC_NS = res.exec_time_ns
    LAST_RESULTS = res

    outs = np.stack([r["out"] for r in res.results])   # (8, {1|128}, C)
    d_total = float(outs.sum(dtype=np.float64))
    n_valid = int(np.sum(label != NUM_CLASSES))

    loss = (LOSS_WEIGHT * ANCHOR_WEIGHT
            * (d_total / M - gbb_mean * n_valid) / max(n_valid, 1))
    return np.float32(loss)

